# revision 23
# baseline (speedup 1.0000x reference)
"""Trainium2 Bass kernel for local (block-sparse) scaled-dot-product attention.

Contract: kernel(**inputs) takes the FULL inputs of the reference
(query/key_in/value [8, 4096, 512] fp32, Wq/Wk/Wv/Wo [512, 512], biases [512])
and returns the FULL output [8, 4096, 512] fp32.

Sharding: data-parallel over batch; batch element b runs on NeuronCore b.

On-chip layout is feature-major ("transposed"): activations live as [feat, t]
so the contraction dim of every matmul is on partitions. The CPU pre-transposes
the inputs/weights (free) and transposes the output back.

Key structure (per core, per t-group of 512 positions):
 - q/k projections feature-major in fp8 (DoubleRowSwInterleave, 2x PE rate);
   v/o projections bf16 (fp8 there fails the accuracy budget).
 - Per block n, scores sT[j, q] over the 128-wide key window [64n-32, 64n+96).
 - Window masks fold into the exp activation as per-partition bias vectors
   applied to q-halves (no mask matmuls).
 - Softmax normalization happens *after* PV: oT_raw = sum_k e_k v_k times the
   PE-broadcast reciprocal denominators (bc2).  bv folds into bo on the CPU
   (bo' = Wo@bv + bo; softmax weights sum to 1), so v needs no bias.
 - Emission is software-pipelined: loads prefetch one group ahead, and the
   output projection of group g is interleaved into the projections of group
   g+1 so the PE never waits on the softmax tail.
"""

import math

import numpy as np
import ml_dtypes

import concourse.bass as bass
import concourse.tile as tile
from concourse import bacc, mybir
from concourse.bass_utils import run_bass_kernel_spmd

# ---- problem constants (hardcoded; must match the reference) ----
B, T, F = 8, 4096, 512
H, DK, DV = 8, 64, 64
CTX = 64          # block size (cq == ck == 64, nb == 64)
NB = T // CTX     # 64 blocks
NEG = -1e20
SCALE = 1.0 / math.sqrt(DK)

TG = 8            # t-groups per core
TT = T // TG      # 512 t positions per group
NB8 = TT // CTX   # 8 blocks per group

DT = mybir.dt.bfloat16
NP_DT = ml_dtypes.bfloat16
F32 = mybir.dt.float32
FP8 = mybir.dt.float8e4
NP_FP8 = ml_dtypes.float8_e4m3

FP8_QK = True  # fp8 DoubleRowSwInterleave for the q/k projections

_CACHED = None


def _build_maskbias():
    """Per-partition fp32 bias vectors for the windowed-softmax masks.

    Scores tile sT[j, q] per block: j in 0..128 indexes keys
    [64n-32, 64n+96), q in 0..64.  exp is evaluated as
    exp(SCALE*s + bias[j]) separately on q-halves:
      A: NEG for j >= 96            (mid/last blocks, q < 32)
      B: NEG for j < 32             (mid/first blocks, q >= 32)
      C: NEG for j < 32 or j >= 96  (block 0 q<32; block 63 q>=32)
    """
    j = np.arange(128)
    mb = np.zeros((3, 128), np.float32)
    mb[0] = NEG * (j >= 96)
    mb[1] = NEG * (j < 32)
    mb[2] = NEG * ((j < 32) | (j >= 96))
    return mb


def _sumsel():
    # ss[p, 8*n8 + m] = 1 if m == n8 (lhsT for per-block column sums)
    s = np.zeros((128, 64), np.float32)
    for n8 in range(8):
        s[:, 8 * n8 + n8] = 1.0
    return s


def _rowsel():
    # rsel[m, 128*n8 + p] = 1 if m == n8 (lhsT to broadcast rs row n8)
    r = np.zeros((8, 1024), np.float32)
    for n8 in range(8):
        r[n8, 128 * n8 : 128 * n8 + 128] = 1.0
    return r


def _build_nc(n_iter=1):
    nc = bacc.Bacc(None, target_bir_lowering=False, debug=False)

    qk_dt = FP8 if FP8_QK else DT
    xq = nc.dram_tensor("xq", [F, T], qk_dt, kind="ExternalInput")
    xk = nc.dram_tensor("xk", [F, T], qk_dt, kind="ExternalInput")
    xv = nc.dram_tensor("xv", [F, T], DT, kind="ExternalInput")
    if FP8_QK:
        # SwInterleave weight layout (see _prep_w8): [p, j, oc, 2m+t]
        wq = nc.dram_tensor("wq", [128, 2, 4, 256], FP8, kind="ExternalInput")
        wk = nc.dram_tensor("wk", [128, 2, 4, 256], FP8, kind="ExternalInput")
    else:
        wq = nc.dram_tensor("wq", [F, F], DT, kind="ExternalInput")  # Wq.T
        wk = nc.dram_tensor("wk", [F, F], DT, kind="ExternalInput")  # Wk.T
    wv = nc.dram_tensor("wv", [F, F], DT, kind="ExternalInput")  # Wv.T
    wo = nc.dram_tensor("wo", [F, F], DT, kind="ExternalInput")  # Wo.T
    bq = nc.dram_tensor("bq", [F], F32, kind="ExternalInput")
    bk = nc.dram_tensor("bk", [F], F32, kind="ExternalInput")
    bo = nc.dram_tensor("bo", [F], F32, kind="ExternalInput")  # Wo@bv + bo
    mb = nc.dram_tensor("mb", [3, 128], F32, kind="ExternalInput")
    ss = nc.dram_tensor("ss", [128, 64], DT, kind="ExternalInput")
    rsel = nc.dram_tensor("rsel", [8, 1024], DT, kind="ExternalInput")
    outd = nc.dram_tensor("out", [F, T], DT, kind="ExternalOutput")

    Exp = mybir.ActivationFunctionType.Exp
    DR = mybir.MatmulPerfMode.DoubleRowSwInterleave

    with tile.TileContext(nc) as tc:
        with (
            tc.tile_pool(name="singles", bufs=1) as singles,
            tc.tile_pool(name="xin", bufs=2) as xin,
            tc.tile_pool(name="proj_out", bufs=2) as pqk,
            tc.tile_pool(name="vpool", bufs=2) as vpool,
            tc.tile_pool(name="epool", bufs=3) as epool,
            tc.tile_pool(name="ypool", bufs=2) as ypool,
            tc.tile_pool(name="opool", bufs=2) as opool,
            tc.tile_pool(name="ps_proj", bufs=2, space="PSUM") as ps_proj,
            tc.tile_pool(name="ps_s", bufs=2, space="PSUM") as ps_s,
            tc.tile_pool(name="ps_r", bufs=2, space="PSUM") as ps_r,
        ):
            # ---- static tiles (loads spread across DMA queues) ----
            if FP8_QK:
                wq_t = singles.tile([128, 2, 4, 256], FP8, tag="wq")
                wk_t = singles.tile([128, 2, 4, 256], FP8, tag="wk")
                nc.sync.dma_start(out=wq_t, in_=wq[:, :, :, :])
                nc.sync.dma_start(out=wk_t, in_=wk[:, :, :, :])
            else:
                wq_t = singles.tile([128, 4, F], DT, tag="wq")
                wk_t = singles.tile([128, 4, F], DT, tag="wk")
                nc.sync.dma_start(
                    out=wq_t, in_=wq.rearrange("(c p) o -> p c o", p=128)
                )
                nc.sync.dma_start(
                    out=wk_t, in_=wk.rearrange("(c p) o -> p c o", p=128)
                )
            wv_t = singles.tile([128, 4, F], DT, tag="wv")
            wo_t = singles.tile([128, 4, F], DT, tag="wo")
            nc.scalar.dma_start(out=wv_t, in_=wv.rearrange("(c p) o -> p c o", p=128))
            nc.scalar.dma_start(out=wo_t, in_=wo.rearrange("(c p) o -> p c o", p=128))
            bq_t = singles.tile([128, 4], F32, tag="bq")
            bk_t = singles.tile([128, 4], F32, tag="bk")
            bo_t = singles.tile([128, 4], F32, tag="bo")
            for bt, bd in ((bq_t, bq), (bk_t, bk), (bo_t, bo)):
                nc.gpsimd.dma_start(out=bt, in_=bd.rearrange("(c p) -> p c", p=128))
            mb_t = singles.tile([128, 3], F32, tag="mb")
            nc.gpsimd.dma_start(out=mb_t, in_=mb.rearrange("k p -> p k"))
            ss_t = singles.tile([128, 64], DT, tag="ss")
            nc.gpsimd.dma_start(out=ss_t, in_=ss[:, :])
            rsel_t = singles.tile([8, 1024], DT, tag="rsel")
            nc.gpsimd.dma_start(out=rsel_t, in_=rsel[:, :])

            xq_r = xq.rearrange("(c p) t -> p c t", p=128)
            xk_r = xk.rearrange("(c p) t -> p c t", p=128)
            xv_r = xv.rearrange("(c p) t -> p c t", p=128)
            out_r = outd.rearrange("(c p) t -> p c t", p=128)

            def qk_proj(ps, w_t, x_s, oc, lo, n):
                # ps[0:128, 0:n] += (W.T chunk).T @ x  over the 4 f-chunks
                if FP8_QK:
                    for j in range(2):
                        nc.tensor.matmul(
                            ps,
                            lhsT=w_t[:, j, oc, :],
                            rhs=x_s[:, 2 * j : 2 * j + 2, lo : lo + n],
                            start=(j == 0),
                            stop=(j == 1),
                            perf_mode=DR,
                        )
                else:
                    for fc in range(4):
                        nc.tensor.matmul(
                            ps,
                            lhsT=w_t[:, fc, oc * 128 : (oc + 1) * 128],
                            rhs=x_s[:, fc, lo : lo + n],
                            start=(fc == 0),
                            stop=(fc == 3),
                        )

            def emit_loads(tg):
                t0 = tg * TT
                xq_s = xin.tile([128, 4, TT], qk_dt, tag="xq")
                nc.sync.dma_start(out=xq_s, in_=xq_r[:, :, t0 : t0 + TT])
                lo, hi = t0 - 32, t0 + TT + 32
                clo, chi = max(lo, 0), min(hi, T)
                xk_s = xin.tile([128, 4, TT + 64], qk_dt, tag="xk")
                xv_s = xin.tile([128, 4, TT + 64], DT, tag="xv")
                for xs, xr in ((xk_s, xk_r), (xv_s, xv_r)):
                    nc.sync.dma_start(
                        out=xs[:, :, clo - lo : chi - lo], in_=xr[:, :, clo:chi]
                    )
                    if clo > lo:
                        nc.vector.memset(xs[:, :, 0 : clo - lo], 0.0)
                    if chi < hi:
                        nc.vector.memset(xs[:, :, TT + 64 - (hi - chi) :], 0.0)
                return xq_s, xk_s, xv_s

            def emit_group(tg, tiles, next_tiles, prev=None):
                t0 = tg * TT
                xq_s, xk_s, xv_s = tiles

                # ---- v projection (t-major), covering [t0-32, t0+544) ----
                # First so the v0s shuffle DMA overlaps the q/k projections.
                # No bias: bv is folded into bo on the CPU.
                v0 = vpool.tile([128, 5, F], DT, tag="v0")
                for tc5 in range(5):
                    m = 128 if tc5 < 4 else 64
                    ps = ps_proj.tile([128, 512], F32, tag="proj")
                    for fc in range(4):
                        nc.tensor.matmul(
                            ps[0:m, :],
                            lhsT=xv_s[:, fc, 128 * tc5 : 128 * tc5 + m],
                            rhs=wv_t[:, fc, :],
                            start=(fc == 0),
                            stop=(fc == 3),
                        )
                    if tc5 % 2 == 1:
                        nc.scalar.copy(out=v0[0:m, tc5, :], in_=ps[0:m, :])
                    else:
                        nc.vector.tensor_copy(out=v0[0:m, tc5, :], in_=ps[0:m, :])
                # shifted copy: v0s covers [t0+32, t0+544), chunk c = rows
                # [64..128) of v0 chunk c plus rows [0..64) of v0 chunk c+1.
                # Issued on gpsimd so they don't queue behind SP input loads.
                v0s = vpool.tile([128, 4, F], DT, tag="v0s")
                nc.gpsimd.dma_start(out=v0s[0:64, :, :], in_=v0[64:128, 0:4, :])
                nc.gpsimd.dma_start(out=v0s[64:128, :, :], in_=v0[0:64, 1:5, :])

                # ---- q/k projections (feature-major), interleaved with the
                # previous group's output projection so the PE stays fed
                # while the fp8 q/k evacuations drain.
                qT = pqk.tile([128, 4, TT], DT, tag="qT")
                kT = pqk.tile([128, 4, TT + 64], DT, tag="kT")
                if prev is not None:
                    yT_prev, t0_prev = prev
                    outsb = opool.tile([128, 4, TT], DT, tag="outsb")
                for oc in range(4):
                    ps = ps_proj.tile([128, 512], F32, tag="proj")
                    qk_proj(ps, wq_t, xq_s, oc, 0, 512)
                    nc.vector.tensor_scalar_add(qT[:, oc, :], ps, bq_t[:, oc : oc + 1])
                    ps = ps_proj.tile([128, 512], F32, tag="proj")
                    qk_proj(ps, wk_t, xk_s, oc, 0, 512)
                    nc.vector.tensor_scalar_add(
                        kT[:, oc, 0:512], ps, bk_t[:, oc : oc + 1]
                    )
                    # k-halo psum in ps_r (idle during projections) so
                    # ps_proj keeps double-buffering q/k.
                    ps2 = ps_r.tile([128, 64], F32, tag="r")
                    qk_proj(ps2, wk_t, xk_s, oc, 512, 64)
                    nc.scalar.add(kT[:, oc, 512:576], ps2, bk_t[:, oc : oc + 1])
                    if prev is not None:
                        pso = ps_s.tile([128, 512], F32, tag="sT")
                        for fc in range(4):
                            nc.tensor.matmul(
                                pso,
                                lhsT=wo_t[:, fc, oc * 128 : (oc + 1) * 128],
                                rhs=yT_prev[:, fc, :],
                                start=(fc == 0),
                                stop=(fc == 3),
                            )
                        nc.scalar.add(outsb[:, oc, :], pso, bo_t[:, oc : oc + 1])
                if prev is not None:
                    nc.gpsimd.dma_start(
                        out=out_r[:, :, t0_prev : t0_prev + TT], in_=outsb
                    )

                # prefetch next group's inputs while attention runs
                if next_tiles is not None:
                    next_tiles.append(emit_loads(tg + 1))

                return qT, kT, v0, v0s

            def emit_pair_scores(tg, proj, hp):
                qT, kT, v0, v0s = proj
                oc = hp
                # Both heads' QK matmuls adjacently: disjoint 64-row
                # contraction groups pack in the PE array.
                sT = ps_s.tile([128, 2, NB8, 64], F32, tag="sT")
                for n8 in range(NB8):
                    for hl in range(2):
                        pb = hl * 64
                        nc.tensor.matmul(
                            sT[:, hl, n8, :],
                            lhsT=kT[pb : pb + 64, oc, 64 * n8 : 64 * n8 + 128],
                            rhs=qT[pb : pb + 64, oc, 64 * n8 : 64 * n8 + 64],
                            start=True,
                            stop=True,
                        )
                # exp with mask-bias on q-halves -> eT (pair tile)
                eT = epool.tile([128, 2, NB8, 64], DT, tag="eT")
                act = nc.scalar.activation
                if tg == 0:
                    # block 0 is a 'first' block: bias C on its q-half 0
                    act(out=eT[:, :, 0, 0:32], in_=sT[:, :, 0, 0:32],
                        func=Exp, scale=SCALE, bias=mb_t[:, 2:3])
                    act(out=eT[:, :, 1:NB8, 0:32], in_=sT[:, :, 1:NB8, 0:32],
                        func=Exp, scale=SCALE, bias=mb_t[:, 0:1])
                    act(out=eT[:, :, :, 32:64], in_=sT[:, :, :, 32:64],
                        func=Exp, scale=SCALE, bias=mb_t[:, 1:2])
                elif tg == TG - 1:
                    # block 63 is a 'last' block: bias C on its q-half 1
                    act(out=eT[:, :, :, 0:32], in_=sT[:, :, :, 0:32],
                        func=Exp, scale=SCALE, bias=mb_t[:, 0:1])
                    act(out=eT[:, :, 0 : NB8 - 1, 32:64],
                        in_=sT[:, :, 0 : NB8 - 1, 32:64],
                        func=Exp, scale=SCALE, bias=mb_t[:, 1:2])
                    act(out=eT[:, :, NB8 - 1, 32:64],
                        in_=sT[:, :, NB8 - 1, 32:64],
                        func=Exp, scale=SCALE, bias=mb_t[:, 2:3])
                else:
                    act(out=eT[:, :, :, 0:32], in_=sT[:, :, :, 0:32],
                        func=Exp, scale=SCALE, bias=mb_t[:, 0:1])
                    act(out=eT[:, :, :, 32:64], in_=sT[:, :, :, 32:64],
                        func=Exp, scale=SCALE, bias=mb_t[:, 1:2])
                return eT

            def emit_pair_tail(proj, yT, hp, eT):
                qT, kT, v0, v0s = proj
                oc = hp
                # paired per-block column sums -> [8, 2, 64]
                sums = ps_r.tile([8, 2, 64], F32, tag="r")
                for n8 in range(NB8):
                    nc.tensor.matmul(
                        sums,
                        lhsT=ss_t[:, 8 * n8 : 8 * n8 + 8],
                        rhs=eT[:, :, n8, :],
                        start=(n8 == 0),
                        stop=(n8 == NB8 - 1),
                    )
                rs = epool.tile([8, 2, 64], DT, tag="rs")
                with nc.allow_low_precision(reason="bf16 softmax denominators"):
                    nc.vector.reciprocal(out=rs, in_=sums)
                # PV on unnormalized e (the reciprocal's DVE latency hides
                # under the PV matmuls)
                oT = ps_r.tile([128, 512], F32, tag="r")
                for hl in range(2):
                    h = 2 * hp + hl
                    pb = hl * 64
                    for n8 in range(NB8):
                        if n8 % 2 == 0:
                            lhsT = v0[:, n8 // 2, 64 * h : 64 * h + 64]
                        else:
                            lhsT = v0s[:, (n8 - 1) // 2, 64 * h : 64 * h + 64]
                        nc.tensor.matmul(
                            oT[pb : pb + 64, 64 * n8 : 64 * n8 + 64],
                            lhsT=lhsT,
                            rhs=eT[:, hl, n8, :],
                            start=True,
                            stop=True,
                            tile_position=(0, pb),
                        )
                # broadcast reciprocals to oT layout: bc2[64*hl+d, 64*n8+q]
                bc2 = ps_r.tile([128, NB8, 64], F32, tag="r")
                for n8 in range(NB8):
                    for hl in range(2):
                        nc.tensor.matmul(
                            bc2[64 * hl : 64 * hl + 64, n8, :],
                            lhsT=rsel_t[:, 128 * n8 : 128 * n8 + 64],
                            rhs=rs[:, hl, :],
                            start=True,
                            stop=True,
                            tile_position=(0, 64 * hl),
                        )
                # normalize after PV: yT = oT * bc2.  The DVE multiply may
                # only take one PSUM operand, so stage bc2 in SBUF via Act.
                bcs = epool.tile([128, NB8, 64], DT, tag="bcs")
                nc.scalar.copy(out=bcs, in_=bc2)
                nc.vector.tensor_mul(
                    yT[:, oc, :], oT, bcs.rearrange("p a b -> p (a b)")
                )

            def emit_attention(tg, proj):
                yT = ypool.tile([128, 4, TT], DT, tag="yT")
                prev = None
                for hp in range(H // 2):
                    eT = emit_pair_scores(tg, proj, hp)
                    if prev is not None:
                        emit_pair_tail(proj, yT, hp - 1, prev)
                    prev = eT
                emit_pair_tail(proj, yT, H // 2 - 1, prev)
                return yT

            def emit_oproj(tg, yT):
                t0 = tg * TT
                outsb = opool.tile([128, 4, TT], DT, tag="outsb")
                for oc in range(4):
                    pso = ps_s.tile([128, 512], F32, tag="sT")
                    for fc in range(4):
                        nc.tensor.matmul(
                            pso,
                            lhsT=wo_t[:, fc, oc * 128 : (oc + 1) * 128],
                            rhs=yT[:, fc, :],
                            start=(fc == 0),
                            stop=(fc == 3),
                        )
                    nc.scalar.add(outsb[:, oc, :], pso, bo_t[:, oc : oc + 1])
                nc.gpsimd.dma_start(out=out_r[:, :, t0 : t0 + TT], in_=outsb)

            def emit_all():
                tiles = emit_loads(0)
                nxt = []
                proj = emit_group(0, tiles, nxt)
                for tg in range(TG):
                    yT = emit_attention(tg, proj)
                    if tg + 1 < TG:
                        tiles = nxt[0]
                        nxt = [] if tg + 2 < TG else None
                        proj = emit_group(tg + 1, tiles, nxt, prev=(yT, tg * TT))
                    else:
                        emit_oproj(tg, yT)

            if n_iter == 1:
                emit_all()
            else:
                with tc.For_i(0, n_iter, 1):
                    emit_all()

    nc.finalize()
    return nc


def _get_nc(n_iter=1):
    global _CACHED
    if _CACHED is None:
        _CACHED = {}
    if n_iter not in _CACHED:
        _CACHED[n_iter] = _build_nc(n_iter)
    return _CACHED[n_iter]


def _prep_w8(W):
    """fp8 DoubleRowSwInterleave weight layout for a [F, F] weight.

    w8[p, j, oc, 2m+t] = W.T[(2j+t)*128 + p, oc*128 + (127-m)]
    (per-partition columns stored as interleaved (tile0, tile1) pairs in
    reversed column order — what the PE's SwInterleave mode consumes).
    """
    WT = np.ascontiguousarray(np.asarray(W, np.float32).T).astype(NP_FP8)
    r = WT.reshape(2, 2, 128, 4, 128)  # (j, t, p, oc, m)
    r = r[:, :, :, :, ::-1]  # reverse m
    w8 = np.stack([r[:, 0], r[:, 1]], axis=-1)  # (j, p, oc, m, t)
    w8 = w8.transpose(1, 0, 2, 3, 4).reshape(128, 2, 4, 256)
    return np.ascontiguousarray(w8)


def _prep_in_maps(query, key_in, value, Wq, bq, Wk, bk, Wv, bv, Wo, bo):
    np_qk = NP_FP8 if FP8_QK else NP_DT
    bo_prime = (
        np.asarray(Wo, np.float32) @ np.asarray(bv, np.float32)
        + np.asarray(bo, np.float32)
    )
    if FP8_QK:
        wq_prep = _prep_w8(Wq)
        wk_prep = _prep_w8(Wk)
    else:
        wq_prep = np.ascontiguousarray(Wq.T).astype(NP_DT)
        wk_prep = np.ascontiguousarray(Wk.T).astype(NP_DT)
    shared = {
        "wq": wq_prep,
        "wk": wk_prep,
        "wv": np.ascontiguousarray(Wv.T).astype(NP_DT),
        "wo": np.ascontiguousarray(Wo.T).astype(NP_DT),
        "bq": np.asarray(bq, np.float32),
        "bk": np.asarray(bk, np.float32),
        "bo": bo_prime,
        "mb": _build_maskbias(),
        "ss": _sumsel().astype(NP_DT),
        "rsel": _rowsel().astype(NP_DT),
    }
    from concurrent.futures import ThreadPoolExecutor

    def _tp(a):
        return np.ascontiguousarray(np.asarray(a, np.float32).T.astype(NP_DT))

    def _tp8(a):
        return np.ascontiguousarray(np.asarray(a, np.float32).T.astype(np_qk))

    with ThreadPoolExecutor(12) as ex:
        xqs = list(ex.map(_tp8, [query[b] for b in range(B)]))
        xks = list(ex.map(_tp8, [key_in[b] for b in range(B)]))
        xvs = list(ex.map(_tp, [value[b] for b in range(B)]))
    in_maps = []
    for b in range(B):
        in_maps.append({"xq": xqs[b], "xk": xks[b], "xv": xvs[b], **shared})
    return in_maps


def run(trace=False, **inputs):
    nc = _get_nc()
    in_maps = _prep_in_maps(**inputs)
    res = run_bass_kernel_spmd(
        nc, in_maps, core_ids=list(range(B)), trace=trace
    )
    out = np.stack(
        [
            np.asarray(res.results[b]["out"]).astype(np.float32).T
            for b in range(B)
        ]
    )
    return out, res


def kernel(**inputs):
    out, _ = run(trace=False, **inputs)
    return out
